# revision 1
# baseline (speedup 1.0000x reference)
"""GQA (grouped-query attention) Trainium2 kernel, tensor-parallel over 8 cores.

Problem: hidden [1,2048,4096] x (Wq[4096,4096], Wk/Wv[4096,1024], Wo[4096,4096])
H=32 query heads, G=8 KV groups, D=128, causal, RoPE (LLaMA rotate-half).

Sharding: core i owns query heads 4i..4i+3 and KV group i (Wq/Wk/Wv column
slices), plus the matching Wo row slice. Each core computes a full [2048,4096]
partial output; the host sums the 8 partials and adds bo.

On-device layout is fully transposed (partition = feature dim) so every matmul
runs with a 512-wide fp32r moving operand at full PE rate:
  phase A: QT/KT/VT = W.T @ X.T   (X.T streamed in 128-row chunks, RoPE fused)
  phase B: S.T = K.T x Q slabs -> exp -> P.T; ctx.T = V.T-chunks @ P.T;
           denominators via ones-row matmul (partition reduction on PE)
  phase C: out = ctx.T-chunks.T @ Wo rows, streamed back to DRAM.
"""

import math

import numpy as np

import concourse.bacc as bacc
import concourse.tile as tile
from concourse import mybir
from concourse import bass_utils

# ---- problem constants (hardcoded per contest contract) ----
S = 2048          # sequence length
HID = 4096        # hidden size
H = 32            # query heads
G = 8             # KV groups
D = 128           # head dim
THETA = 10000.0
NCORES = 8
RH = H // NCORES      # query heads per core = 4
HD_LOC = RH * D       # local head width = 512

P = 128               # partitions
SQT = 512             # seq tile width (moving operand)
NSQT = S // SQT       # 4
NCH = HID // P        # 32 contraction chunks
NSKB = S // P         # 16 key blocks

f32 = mybir.dt.float32
f32r = mybir.dt.float32r

_SCALE = 1.0 / math.sqrt(D)


def _round_f32r(x: np.ndarray) -> np.ndarray:
    """Round fp32 to fp32r (11-bit mantissa), nearest-even; matches PE input fmt."""
    u = np.ascontiguousarray(x, dtype=np.float32).view(np.uint32)
    low = u & np.uint32(0xFFF)
    up = (low > np.uint32(0x800)) | (
        (low == np.uint32(0x800)) & (((u >> np.uint32(12)) & np.uint32(1)) == 1)
    )
    out = (u & np.uint32(0xFFFFF000)) + np.where(up, np.uint32(0x1000), np.uint32(0))
    return out.view(np.float32)


def _host_consts():
    """RoPE tables (transposed layout) + sliding causal mask tile."""
    half = D // 2
    inv_freq = 1.0 / (THETA ** (np.arange(half, dtype=np.float64) / half))
    ang = np.arange(S, dtype=np.float64)[None, :] * inv_freq[:, None]  # [64, S]
    cos = np.cos(ang)
    sin = np.sin(ang)
    cos_t = np.concatenate([cos, cos], axis=0).astype(np.float32)        # [128, S]
    sin_t = np.concatenate([sin, -sin], axis=0).astype(np.float32)       # [128, S] (swapped-half layout)
    # maskt[i, c] = 1.0 if i <= c - 512 else 0 ; diagonal tile slice [512-t, 1024-t)
    i = np.arange(P)[:, None]
    c = np.arange(2 * SQT)[None, :]
    maskt = (i <= c - SQT).astype(np.float32)                            # [128, 1024]
    ones_pp = np.ones((P, P), np.float32)
    return cos_t, sin_t, maskt, ones_pp


def build_kernel() -> bacc.Bacc:
    nc = bacc.Bacc("TRN2", target_bir_lowering=False, debug=False)

    xt = nc.dram_tensor("xt", [HID, S], f32r, kind="ExternalInput")
    wq = nc.dram_tensor("wq", [HID, HD_LOC], f32r, kind="ExternalInput")
    wk = nc.dram_tensor("wk", [HID, D], f32r, kind="ExternalInput")
    wv = nc.dram_tensor("wv", [HID, D], f32r, kind="ExternalInput")
    wo = nc.dram_tensor("wo", [HD_LOC, HID], f32r, kind="ExternalInput")
    bq = nc.dram_tensor("bq", [RH, D], f32, kind="ExternalInput")
    bk = nc.dram_tensor("bk", [1, D], f32, kind="ExternalInput")
    bv = nc.dram_tensor("bv", [1, D], f32, kind="ExternalInput")
    cos_d = nc.dram_tensor("cos_t", [P, S], f32, kind="ExternalInput")
    sin_d = nc.dram_tensor("sin_t", [P, S], f32, kind="ExternalInput")
    mask_d = nc.dram_tensor("maskt", [P, 2 * SQT], f32r, kind="ExternalInput")
    onec_d = nc.dram_tensor("ones_pp", [P, P], f32r, kind="ExternalInput")
    out_d = nc.dram_tensor("out_partial", [S, HID], f32, kind="ExternalOutput")

    Copy = mybir.ActivationFunctionType.Copy
    Exp = mybir.ActivationFunctionType.Exp

    with tile.TileContext(nc) as tc:
        with tc.tile_pool(name="consts", bufs=1) as consts, \
             tc.tile_pool(name="qkv", bufs=1) as qkv:
            cos_sb = consts.tile([P, S], f32)
            sin_sb = consts.tile([P, S], f32)
            mask_sb = consts.tile([P, 2 * SQT], f32r)
            onec_sb = consts.tile([P, P], f32r)
            bq_sb = consts.tile([P, RH], f32)
            bk_sb = consts.tile([P, 1], f32)
            bv_sb = consts.tile([P, 1], f32)
            nc.sync.dma_start(out=cos_sb, in_=cos_d[:])
            nc.sync.dma_start(out=sin_sb, in_=sin_d[:])
            nc.sync.dma_start(out=mask_sb, in_=mask_d[:])
            nc.sync.dma_start(out=onec_sb, in_=onec_d[:])
            nc.sync.dma_start(out=bq_sb, in_=bq[:].rearrange("o p -> p o"))
            nc.sync.dma_start(out=bk_sb, in_=bk[:].rearrange("o p -> p o"))
            nc.sync.dma_start(out=bv_sb, in_=bv[:].rearrange("o p -> p o"))

            # persistent transposed activations (fp32r, matmul-ready)
            qt_sb = [qkv.tile([P, S], f32r, tag=f"qt{ob}", name=f"qt_sb{ob}") for ob in range(RH)]
            kt_sb = qkv.tile([P, S], f32r, tag="kt")
            v_sb = qkv.tile([P, S], f32r, tag="v")   # V in [sk%128, skb*128+d] layout

            # ---------------- phase A: projections + RoPE ----------------
            with tc.tile_pool(name="wa", bufs=1) as wa, \
                 tc.tile_pool(name="xts", bufs=4) as xts, \
                 tc.tile_pool(name="tmpa", bufs=2) as tmpa, \
                 tc.tile_pool(name="psa", bufs=1, space="PSUM") as psa:
                wq_sb = wa.tile([P, NCH, HD_LOC], f32r)
                wk_sb = wa.tile([P, NCH, D], f32r)
                wv_sb = wa.tile([P, NCH, D], f32r)
                # per-chunk weight loads: first matmul starts after ~1us, not
                # after the whole 8.4MB lands
                wq_r = wq[:].rearrange("(c p) o -> p c o", p=P)
                wk_r = wk[:].rearrange("(c p) o -> p c o", p=P)
                wv_r = wv[:].rearrange("(c p) o -> p c o", p=P)
                for c in range(NCH):
                    nc.sync.dma_start(out=wq_sb[:, c], in_=wq_r[:, c])
                    if c % 8 == 0:
                        cs = slice(c, c + 8)
                        nc.sync.dma_start(out=wk_sb[:, cs], in_=wk_r[:, cs])
                        nc.sync.dma_start(out=wv_sb[:, cs], in_=wv_r[:, cs])
                ident = consts.tile([P, P], f32)
                from concourse.masks import make_identity
                make_identity(nc, ident)

                for st in range(NSQT):
                    sq = slice(st * SQT, (st + 1) * SQT)
                    q_ps = [psa.tile([P, SQT], f32, tag=f"q_ps{ob}", name=f"q_ps{ob}") for ob in range(RH)]
                    k_ps = psa.tile([P, SQT], f32, tag="k_ps")
                    v_ps = psa.tile([P, SQT], f32, tag="v_ps")
                    for c in range(NCH):
                        xtc = xts.tile([P, SQT], f32r, tag="xtc")
                        # scalar-engine HWDGE queue: keeps xt stream off the
                        # sync sequencer that issues the weight loads
                        nc.scalar.dma_start(out=xtc, in_=xt[c * P:(c + 1) * P, sq])
                        for ob in range(RH):
                            nc.tensor.matmul(q_ps[ob], lhsT=wq_sb[:, c, ob * P:(ob + 1) * P],
                                             rhs=xtc, start=(c == 0), stop=(c == NCH - 1))
                        nc.tensor.matmul(k_ps, lhsT=wk_sb[:, c, :], rhs=xtc,
                                         start=(c == 0), stop=(c == NCH - 1))
                        nc.tensor.matmul(v_ps, lhsT=wv_sb[:, c, :], rhs=xtc,
                                         start=(c == 0), stop=(c == NCH - 1))

                    # V first (feeds PE transposes), then K (gates phase B), then Q
                    vb = tmpa.tile([P, SQT], f32, tag="qb")
                    nc.vector.tensor_scalar_add(vb, v_ps, bv_sb)
                    for j in range(SQT // P):
                        vt_ps = psa.tile([P, P], f32, tag="vt_ps", bufs=2)
                        nc.tensor.transpose(vt_ps, vb[:, j * P:(j + 1) * P], ident)
                        skb = st * (SQT // P) + j
                        nc.vector.tensor_copy(v_sb[:, skb * P:(skb + 1) * P], vt_ps)
                    kb = tmpa.tile([P, SQT], f32, tag="qb")
                    nc.vector.tensor_scalar_add(kb, k_ps, bk_sb)
                    t2 = tmpa.tile([P, SQT], f32, tag="t2")
                    nc.vector.tensor_mul(t2[0:64], kb[64:128], sin_sb[64:128, sq])
                    nc.vector.tensor_mul(t2[64:128], kb[0:64], sin_sb[0:64, sq])
                    t3 = tmpa.tile([P, SQT], f32, tag="t3")
                    nc.vector.tensor_mul(t3, kb, cos_sb[:, sq])
                    nc.vector.tensor_add(kt_sb[:, sq], t3, t2)
                    for ob in range(RH):
                        qb = tmpa.tile([P, SQT], f32, tag="qb")
                        nc.vector.tensor_scalar_add(qb, q_ps[ob], bq_sb[:, ob:ob + 1])
                        t2 = tmpa.tile([P, SQT], f32, tag="t2")
                        nc.vector.tensor_mul(t2[0:64], qb[64:128], sin_sb[64:128, sq])
                        nc.vector.tensor_mul(t2[64:128], qb[0:64], sin_sb[0:64, sq])
                        t3 = tmpa.tile([P, SQT], f32, tag="t3")
                        nc.vector.tensor_mul(t3, qb, cos_sb[:, sq])
                        nc.vector.tensor_add(qt_sb[ob][:, sq], t3, t2)

            # ---------------- phases B+C (interleaved, C trails B by one tile) ----------------
            with tc.tile_pool(name="ctxp", bufs=1) as ctxp, \
                 tc.tile_pool(name="woc", bufs=1) as woc, \
                 tc.tile_pool(name="outs", bufs=6) as outsp, \
                 tc.tile_pool(name="ptp", bufs=6) as ptp, \
                 tc.tile_pool(name="tmpb", bufs=3) as tmpb, \
                 tc.tile_pool(name="psbc", bufs=1, space="PSUM") as psbc:
                ctx_sb = [ctxp.tile([P, S], f32r, tag=f"ctx{hb}", name=f"ctx_sb{hb}") for hb in range(RH)]
                wo_sb = woc.tile([P, RH, HID], f32r)
                nc.sync.dma_start(out=wo_sb, in_=wo[:].rearrange("(h p) e -> p h e", p=P))

                def emit_B(st):
                    sq = slice(st * SQT, (st + 1) * SQT)
                    nblk = 4 * st + 4
                    for hb in range(RH):
                        ctx_ps = psbc.tile([P, SQT], f32, tag="ctx_ps", bufs=2, name="ctx_ps")
                        den_ps = psbc.tile([P, SQT], f32, tag="den_ps", bufs=1, name="den_ps")
                        for skb in range(nblk):
                            st_ps = psbc.tile([P, SQT], f32, tag="st_ps", bufs=2, name="st_ps")
                            nc.tensor.matmul(st_ps, lhsT=kt_sb[:, skb * P:(skb + 1) * P],
                                             rhs=qt_sb[hb][:, sq], start=True, stop=True)
                            pt = ptp.tile([P, SQT], f32r, tag="pt", name="pt")
                            nc.scalar.activation(out=pt, in_=st_ps, func=Exp, scale=_SCALE)
                            t = skb * P - st * SQT
                            if t >= 0:  # diagonal band -> causal mask (multiplicative)
                                nc.vector.tensor_mul(pt, pt, mask_sb[:, SQT - t: 2 * SQT - t])
                            nc.tensor.matmul(ctx_ps, lhsT=v_sb[:, skb * P:(skb + 1) * P],
                                             rhs=pt, start=(skb == 0), stop=(skb == nblk - 1))
                            nc.tensor.matmul(den_ps, lhsT=onec_sb, rhs=pt,
                                             start=(skb == 0), stop=(skb == nblk - 1))
                        den_sb = tmpb.tile([P, SQT], f32, tag="den_sb", name="den_sb")
                        nc.scalar.copy(den_sb, den_ps)
                        recip_sb = tmpb.tile([P, SQT], f32r, tag="recip", name="recip_sb")
                        with nc.allow_low_precision(reason="softmax denom fp32r feed"):
                            nc.vector.reciprocal(recip_sb, den_sb)
                        nc.vector.tensor_mul(ctx_sb[hb][:, sq], ctx_ps, recip_sb)

                def emit_C(st):
                    for sqb in range(st * (SQT // P), (st + 1) * (SQT // P)):
                        for g in range(4):  # pairs of output column tiles share lhsT loads
                            o_ps = [psbc.tile([P, SQT], f32, tag="o_ps", name=f"o_ps{e}", bufs=3)
                                    for e in range(2)]
                            for hc in range(RH):
                                for e in range(2):
                                    et = 2 * g + e
                                    nc.tensor.matmul(
                                        o_ps[e],
                                        lhsT=ctx_sb[hc][:, sqb * P:(sqb + 1) * P],
                                        rhs=wo_sb[:, hc, et * SQT:(et + 1) * SQT],
                                        start=(hc == 0), stop=(hc == RH - 1))
                            for e in range(2):
                                et = 2 * g + e
                                o_sb = outsp.tile([P, SQT], f32, tag="o_sb", name="o_sb")
                                # DVE copy: keeps ACT free for the exp stream in
                                # the B-overlapped C sections
                                nc.vector.tensor_copy(o_sb, o_ps[e])
                                nc.sync.dma_start(
                                    out=out_d[sqb * P:(sqb + 1) * P, et * SQT:(et + 1) * SQT],
                                    in_=o_sb)

                # C trails B by one seq-tile so the wo DMA (which reuses the
                # freed wq space) lands before the first C matmul needs it.
                emit_B(0)
                emit_B(1)
                emit_C(0)
                emit_B(2)
                emit_C(1)
                emit_B(3)
                emit_C(2)
                emit_C(3)

    nc.compile()
    return nc


_CACHE = {}


def _get_kernel():
    if "nc" not in _CACHE:
        _CACHE["nc"] = build_kernel()
    return _CACHE["nc"]


def kernel(hidden_states, Wq, bq, Wk, bk, Wv, bv, Wo, bo, _trace=False, _trace_kwargs=None):
    hs = np.asarray(hidden_states, dtype=np.float32)
    B = hs.shape[0]
    assert hs.shape == (B, S, HID) and B == 1
    x = hs.reshape(S, HID)

    xt_r = _round_f32r(x.T)                           # [HID, S]
    cos_t, sin_t, maskt, ones_pp = _host_consts()

    in_maps = []
    for i in range(NCORES):
        qs = slice(i * HD_LOC, (i + 1) * HD_LOC)
        ks = slice(i * D, (i + 1) * D)
        in_maps.append({
            "xt": xt_r,
            "wq": _round_f32r(np.asarray(Wq)[:, qs]),
            "wk": _round_f32r(np.asarray(Wk)[:, ks]),
            "wv": _round_f32r(np.asarray(Wv)[:, ks]),
            "wo": _round_f32r(np.asarray(Wo)[qs, :]),
            "bq": np.ascontiguousarray(np.asarray(bq, dtype=np.float32)[qs].reshape(RH, D)),
            "bk": np.ascontiguousarray(np.asarray(bk, dtype=np.float32)[ks].reshape(1, D)),
            "bv": np.ascontiguousarray(np.asarray(bv, dtype=np.float32)[ks].reshape(1, D)),
            "cos_t": cos_t,
            "sin_t": sin_t,
            "maskt": maskt,  # exact 0/1, f32r-safe
            "ones_pp": _round_f32r(ones_pp),
            })

    nc = _get_kernel()
    res = bass_utils.run_bass_kernel_spmd(
        nc, in_maps, core_ids=list(range(NCORES)),
        trace=_trace, **(_trace_kwargs or {}))

    acc = np.zeros((S, HID), dtype=np.float64)
    for i in range(NCORES):
        acc += res.results[i]["out_partial"].astype(np.float64)
    acc += np.asarray(bo, dtype=np.float64)[None, :]
    out = acc.astype(np.float32).reshape(1, S, HID)
    if _trace:
        return out, res
    return out



# revision 5
# speedup vs baseline: 1.1896x; 1.1896x over previous
"""GQA (grouped-query attention) Trainium2 kernel, tensor-parallel over 8 cores.

Problem: hidden [1,2048,4096] x (Wq[4096,4096], Wk/Wv[4096,1024], Wo[4096,4096])
H=32 query heads, G=8 KV groups, D=128, causal, RoPE (LLaMA rotate-half).

Sharding: core i owns query heads 4i..4i+3 and KV group i (Wq/Wk/Wv column
slices), plus the matching Wo row slice. Each core computes a full [2048,4096]
partial output; the host sums the 8 partials and adds bo.

On-device layout is fully transposed (partition = feature dim); all matmul
operands are bf16 (1 cycle/row on the PE at any width), PSUM accumulation fp32:
  phase A: QT/KT/VT = W.T @ X.T   (X.T streamed per-chunk; per seq-tile the
           K and V matmuls run first, interleaved with the xt DMA stream, then
           the 4 Q heads — so the PSUM drain of one tile overlaps the next
           tile's matmuls and the PE never waits on the DVE RoPE chain)
  phase B: S.T = K.T x Q slabs -> exp -> P.T; ctx.T = V.T-chunks @ P.T;
           denominators via ones-row matmul (partition reduction on PE)
  phase C: out = ctx.T-chunks.T @ Wo rows, streamed back to DRAM.
"""

import math

import ml_dtypes
import numpy as np

import concourse.bacc as bacc
import concourse.tile as tile
from concourse import mybir
from concourse import bass_utils

# ---- problem constants (hardcoded per contest contract) ----
S = 2048          # sequence length
HID = 4096        # hidden size
H = 32            # query heads
G = 8             # KV groups
D = 128           # head dim
THETA = 10000.0
NCORES = 8
RH = H // NCORES      # query heads per core = 4
HD_LOC = RH * D       # local head width = 512

P = 128               # partitions
SQT = 512             # seq tile width (moving operand)
NSQT = S // SQT       # 4
NCH = HID // P        # 32 contraction chunks
NSKB = S // P         # 16 key blocks

f32 = mybir.dt.float32
bf16 = mybir.dt.bfloat16

BF16 = ml_dtypes.bfloat16

_SCALE = 1.0 / math.sqrt(D)


def _host_consts():
    """RoPE tables (transposed layout) + sliding causal mask tile."""
    half = D // 2
    inv_freq = 1.0 / (THETA ** (np.arange(half, dtype=np.float64) / half))
    ang = np.arange(S, dtype=np.float64)[None, :] * inv_freq[:, None]  # [64, S]
    cos = np.cos(ang)
    sin = np.sin(ang)
    cos_t = np.concatenate([cos, cos], axis=0).astype(np.float32)        # [128, S]
    sin_t = np.concatenate([sin, -sin], axis=0).astype(np.float32)       # [128, S] (swapped-half layout)
    # maskt[i, c] = 1.0 if i <= c - 512 else 0 ; diagonal tile slice [512-t, 1024-t)
    i = np.arange(P)[:, None]
    c = np.arange(2 * SQT)[None, :]
    maskt = (i <= c - SQT).astype(BF16)                                  # [128, 1024]
    ones_pp = np.ones((P, P), BF16)
    return cos_t, sin_t, maskt, ones_pp


def build_kernel() -> bacc.Bacc:
    nc = bacc.Bacc("TRN2", target_bir_lowering=False, debug=False)

    xt = nc.dram_tensor("xt", [HID, S], bf16, kind="ExternalInput")
    wq = nc.dram_tensor("wq", [HID, HD_LOC], bf16, kind="ExternalInput")
    wk = nc.dram_tensor("wk", [HID, D], bf16, kind="ExternalInput")
    wv = nc.dram_tensor("wv", [HID, D], bf16, kind="ExternalInput")
    wo = nc.dram_tensor("wo", [HD_LOC, HID], bf16, kind="ExternalInput")
    bq = nc.dram_tensor("bq", [RH, D], f32, kind="ExternalInput")
    bk = nc.dram_tensor("bk", [1, D], f32, kind="ExternalInput")
    bv = nc.dram_tensor("bv", [1, D], f32, kind="ExternalInput")
    cos_d = nc.dram_tensor("cos_t", [P, S], f32, kind="ExternalInput")
    sin_d = nc.dram_tensor("sin_t", [P, S], f32, kind="ExternalInput")
    mask_d = nc.dram_tensor("maskt", [P, 2 * SQT], bf16, kind="ExternalInput")
    onec_d = nc.dram_tensor("ones_pp", [P, P], bf16, kind="ExternalInput")
    out_d = nc.dram_tensor("out_partial", [S, HID], f32, kind="ExternalOutput")

    Exp = mybir.ActivationFunctionType.Exp

    with tile.TileContext(nc) as tc:
        with tc.tile_pool(name="consts", bufs=1) as consts, \
             tc.tile_pool(name="qkv", bufs=1) as qkv:
            cos_sb = consts.tile([P, S], f32)
            sin_sb = consts.tile([P, S], f32)
            mask_sb = consts.tile([P, 2 * SQT], bf16)
            onec_sb = consts.tile([P, P], bf16)
            bq_sb = consts.tile([P, RH], f32)
            bk_sb = consts.tile([P, 1], f32)
            bv_sb = consts.tile([P, 1], f32)

            # persistent transposed activations (bf16, matmul-ready)
            qt_sb = [qkv.tile([P, S], bf16, tag=f"qt{ob}", name=f"qt_sb{ob}") for ob in range(RH)]
            kt_sb = qkv.tile([P, S], bf16, tag="kt")
            v_sb = qkv.tile([P, S], bf16, tag="v")   # V in [sk%128, skb*128+d] layout

            # ---------------- phase A: projections + RoPE ----------------
            with tc.tile_pool(name="wa", bufs=1) as wa, \
                 tc.tile_pool(name="xts", bufs=48) as xts, \
                 tc.tile_pool(name="tmpa", bufs=2) as tmpa, \
                 tc.tile_pool(name="psa", bufs=1, space="PSUM") as psa:
                wq_sb = wa.tile([P, NCH, HD_LOC], bf16)
                wk_sb = wa.tile([P, NCH, D], bf16)
                wv_sb = wa.tile([P, NCH, D], bf16)
                wq_r = wq[:].rearrange("(c p) o -> p c o", p=P)
                wk_r = wk[:].rearrange("(c p) o -> p c o", p=P)
                wv_r = wv[:].rearrange("(c p) o -> p c o", p=P)
                # DMA order on the sync queue: wk/wv slabs (needed first by
                # the K/V matmuls), the first wq chunks, then consts (first
                # needed by the st=0 RoPE drain ~15us in), then the rest of wq.
                for c in range(0, NCH, 8):
                    cs = slice(c, c + 8)
                    nc.sync.dma_start(out=wk_sb[:, cs], in_=wk_r[:, cs])
                    nc.sync.dma_start(out=wv_sb[:, cs], in_=wv_r[:, cs])
                for c in range(8):
                    nc.sync.dma_start(out=wq_sb[:, c], in_=wq_r[:, c])
                nc.sync.dma_start(out=cos_sb, in_=cos_d[:])
                nc.sync.dma_start(out=sin_sb, in_=sin_d[:])
                nc.sync.dma_start(out=mask_sb, in_=mask_d[:])
                nc.sync.dma_start(out=onec_sb, in_=onec_d[:])
                nc.sync.dma_start(out=bq_sb, in_=bq[:].rearrange("o p -> p o"))
                nc.sync.dma_start(out=bk_sb, in_=bk[:].rearrange("o p -> p o"))
                nc.sync.dma_start(out=bv_sb, in_=bv[:].rearrange("o p -> p o"))
                for c in range(8, NCH):
                    nc.sync.dma_start(out=wq_sb[:, c], in_=wq_r[:, c])
                ident = consts.tile([P, P], f32)
                from concourse.masks import make_identity
                make_identity(nc, ident)

                for st in range(NSQT):
                    sq = slice(st * SQT, (st + 1) * SQT)
                    # PSUM bank order: k, v first (phase B's score tiles will
                    # land on these banks, and they drain first), then q heads.
                    k_ps = psa.tile([P, SQT], f32, tag="k_ps")
                    v_ps = psa.tile([P, SQT], f32, tag="v_ps")
                    q_ps = [psa.tile([P, SQT], f32, tag=f"q_ps{ob}", name=f"q_ps{ob}") for ob in range(RH)]
                    # xt chunks for this seq tile; scalar-engine HWDGE queue
                    # keeps the stream off the weight-load sync sequencer
                    xtc = []
                    for c in range(NCH):
                        t = xts.tile([P, SQT], bf16, tag="xtc")
                        nc.scalar.dma_start(out=t, in_=xt[c * P:(c + 1) * P, sq])
                        xtc.append(t)
                    # K/V matmul pairs first: they pace the st=0 DMA stream
                    # and their PSUM drain (DVE) gates the next seq tile.
                    for c in range(NCH):
                        nc.tensor.matmul(k_ps, lhsT=wk_sb[:, c, :], rhs=xtc[c],
                                         start=(c == 0), stop=(c == NCH - 1))
                        nc.tensor.matmul(v_ps, lhsT=wv_sb[:, c, :], rhs=xtc[c],
                                         start=(c == 0), stop=(c == NCH - 1))
                    # K drain (DVE) overlaps the q matmuls below
                    kb = tmpa.tile([P, SQT], f32, tag="qb")
                    nc.vector.tensor_scalar_add(kb, k_ps, bk_sb)
                    t2 = tmpa.tile([P, SQT], f32, tag="t2")
                    nc.vector.tensor_mul(t2[0:64], kb[64:128], sin_sb[64:128, sq])
                    nc.vector.tensor_mul(t2[64:128], kb[0:64], sin_sb[0:64, sq])
                    t3 = tmpa.tile([P, SQT], f32, tag="t3")
                    nc.vector.tensor_mul(t3, kb, cos_sb[:, sq])
                    nc.vector.tensor_add(kt_sb[:, sq], t3, t2)
                    vb = tmpa.tile([P, SQT], f32, tag="vb")
                    nc.vector.tensor_scalar_add(vb, v_ps, bv_sb)

                    for ob in range(RH):
                        for c in range(NCH):
                            nc.tensor.matmul(q_ps[ob], lhsT=wq_sb[:, c, ob * P:(ob + 1) * P],
                                             rhs=xtc[c], start=(c == 0), stop=(c == NCH - 1))
                        qb = tmpa.tile([P, SQT], f32, tag="qb")
                        nc.vector.tensor_scalar_add(qb, q_ps[ob], bq_sb[:, ob:ob + 1])
                        t2 = tmpa.tile([P, SQT], f32, tag="t2")
                        nc.vector.tensor_mul(t2[0:64], qb[64:128], sin_sb[64:128, sq])
                        nc.vector.tensor_mul(t2[64:128], qb[0:64], sin_sb[0:64, sq])
                        t3 = tmpa.tile([P, SQT], f32, tag="t3")
                        nc.vector.tensor_mul(t3, qb, cos_sb[:, sq])
                        nc.vector.tensor_add(qt_sb[ob][:, sq], t3, t2)

                    # V transposes at the tile tail (PE), copies on DVE
                    for j in range(SQT // P):
                        vt_ps = psa.tile([P, P], f32, tag="vt_ps", bufs=2)
                        nc.tensor.transpose(vt_ps, vb[:, j * P:(j + 1) * P], ident)
                        skb = st * (SQT // P) + j
                        nc.vector.tensor_copy(v_sb[:, skb * P:(skb + 1) * P], vt_ps)

            # ---------------- phases B+C (interleaved, C trails B by one tile) ----------------
            with tc.tile_pool(name="ctxp", bufs=1) as ctxp, \
                 tc.tile_pool(name="woc", bufs=1) as woc, \
                 tc.tile_pool(name="outs", bufs=6) as outsp, \
                 tc.tile_pool(name="ptp", bufs=6) as ptp, \
                 tc.tile_pool(name="tmpb", bufs=3) as tmpb, \
                 tc.tile_pool(name="psbc", bufs=1, space="PSUM") as psbc:
                ctx_sb = [ctxp.tile([P, S], bf16, tag=f"ctx{hb}", name=f"ctx_sb{hb}") for hb in range(RH)]
                wo_sb = woc.tile([P, RH, HID], bf16)
                nc.sync.dma_start(out=wo_sb, in_=wo[:].rearrange("(h p) e -> p h e", p=P))

                # PSUM pool allocation order maps st_ps onto the k/v banks
                # (drained first at the end of phase A), ctx/den onto q0/q1.
                st_pool = [psbc.tile([P, SQT], f32, tag="st_ps", bufs=2, name="st_ps")
                           for _ in range(2)]

                def emit_B(st):
                    sq = slice(st * SQT, (st + 1) * SQT)
                    nblk = 4 * st + 4
                    for hb in range(RH):
                        ctx_ps = psbc.tile([P, SQT], f32, tag="ctx_ps", bufs=1, name="ctx_ps")
                        den_ps = psbc.tile([P, SQT], f32, tag="den_ps", bufs=1, name="den_ps")
                        for skb in range(nblk):
                            st_ps = psbc.tile([P, SQT], f32, tag="st_ps", bufs=2, name="st_ps")
                            nc.tensor.matmul(st_ps, lhsT=kt_sb[:, skb * P:(skb + 1) * P],
                                             rhs=qt_sb[hb][:, sq], start=True, stop=True)
                            pt = ptp.tile([P, SQT], bf16, tag="pt", name="pt")
                            nc.scalar.activation(out=pt, in_=st_ps, func=Exp, scale=_SCALE)
                            t = skb * P - st * SQT
                            if t >= 0:  # diagonal band -> causal mask (multiplicative)
                                nc.vector.tensor_mul(pt, pt, mask_sb[:, SQT - t: 2 * SQT - t])
                            nc.tensor.matmul(ctx_ps, lhsT=v_sb[:, skb * P:(skb + 1) * P],
                                             rhs=pt, start=(skb == 0), stop=(skb == nblk - 1))
                            nc.tensor.matmul(den_ps, lhsT=onec_sb, rhs=pt,
                                             start=(skb == 0), stop=(skb == nblk - 1))
                        den_sb = tmpb.tile([P, SQT], f32, tag="den_sb", name="den_sb")
                        nc.scalar.copy(den_sb, den_ps)
                        recip_sb = tmpb.tile([P, SQT], f32, tag="recip", name="recip_sb")
                        nc.vector.reciprocal(recip_sb, den_sb)
                        nc.vector.tensor_mul(ctx_sb[hb][:, sq], ctx_ps, recip_sb)

                def emit_C(st):
                    for sqb in range(st * (SQT // P), (st + 1) * (SQT // P)):
                        for g in range(4):  # pairs of output column tiles share lhsT loads
                            o_ps = [psbc.tile([P, SQT], f32, tag="o_ps", name=f"o_ps{e}", bufs=3)
                                    for e in range(2)]
                            for hc in range(RH):
                                for e in range(2):
                                    et = 2 * g + e
                                    nc.tensor.matmul(
                                        o_ps[e],
                                        lhsT=ctx_sb[hc][:, sqb * P:(sqb + 1) * P],
                                        rhs=wo_sb[:, hc, et * SQT:(et + 1) * SQT],
                                        start=(hc == 0), stop=(hc == RH - 1))
                            for e in range(2):
                                et = 2 * g + e
                                o_sb = outsp.tile([P, SQT], f32, tag="o_sb", name="o_sb")
                                # DVE copy: keeps ACT free for the exp stream in
                                # the B-overlapped C sections
                                nc.vector.tensor_copy(o_sb, o_ps[e])
                                nc.sync.dma_start(
                                    out=out_d[sqb * P:(sqb + 1) * P, et * SQT:(et + 1) * SQT],
                                    in_=o_sb)

                # C trails B by one seq-tile so the wo DMA (which reuses the
                # freed wq space) lands before the first C matmul needs it.
                emit_B(0)
                emit_B(1)
                emit_C(0)
                emit_B(2)
                emit_C(1)
                emit_B(3)
                emit_C(2)
                emit_C(3)

    nc.compile()
    return nc


_CACHE = {}


def _get_kernel():
    if "nc" not in _CACHE:
        _CACHE["nc"] = build_kernel()
    return _CACHE["nc"]


def kernel(hidden_states, Wq, bq, Wk, bk, Wv, bv, Wo, bo, _trace=False, _trace_kwargs=None):
    hs = np.asarray(hidden_states, dtype=np.float32)
    B = hs.shape[0]
    assert hs.shape == (B, S, HID) and B == 1
    x = hs.reshape(S, HID)

    xt_b = np.ascontiguousarray(x.T).astype(BF16)     # [HID, S]
    cos_t, sin_t, maskt, ones_pp = _host_consts()

    in_maps = []
    for i in range(NCORES):
        qs = slice(i * HD_LOC, (i + 1) * HD_LOC)
        ks = slice(i * D, (i + 1) * D)
        in_maps.append({
            "xt": xt_b,
            "wq": np.ascontiguousarray(np.asarray(Wq, dtype=np.float32)[:, qs]).astype(BF16),
            "wk": np.ascontiguousarray(np.asarray(Wk, dtype=np.float32)[:, ks]).astype(BF16),
            "wv": np.ascontiguousarray(np.asarray(Wv, dtype=np.float32)[:, ks]).astype(BF16),
            "wo": np.ascontiguousarray(np.asarray(Wo, dtype=np.float32)[qs, :]).astype(BF16),
            "bq": np.ascontiguousarray(np.asarray(bq, dtype=np.float32)[qs].reshape(RH, D)),
            "bk": np.ascontiguousarray(np.asarray(bk, dtype=np.float32)[ks].reshape(1, D)),
            "bv": np.ascontiguousarray(np.asarray(bv, dtype=np.float32)[ks].reshape(1, D)),
            "cos_t": cos_t,
            "sin_t": sin_t,
            "maskt": maskt,
            "ones_pp": ones_pp,
            })

    nc = _get_kernel()
    res = bass_utils.run_bass_kernel_spmd(
        nc, in_maps, core_ids=list(range(NCORES)),
        trace=_trace, **(_trace_kwargs or {}))

    acc = np.zeros((S, HID), dtype=np.float64)
    for i in range(NCORES):
        acc += res.results[i]["out_partial"].astype(np.float64)
    acc += np.asarray(bo, dtype=np.float64)[None, :]
    out = acc.astype(np.float32).reshape(1, S, HID)
    if _trace:
        return out, res
    return out
